# revision 51
# baseline (speedup 1.0000x reference)
"""Epipolar (KNN-sparse) attention on 8 Trainium2 NeuronCores — final.

Sharding (query-parallel): 8 cores = 2 batches x 4 query-quarters.
Each core handles 1024 queries x ALL 8 heads; gathered kv rows are
full-width [k(512)||v(512)] = 2KB (minimum SWDGE descriptor count —
splitting k/v gathers doubles gpsimd desc-gen time, measured +100us).

Design (480us v2 baseline -> ~405us):
  - phase 1 (kv projection of the full target table into kv_dram) loads
    its inputs per contraction chunk, k-chain-first, so the PE matmuls
    start after ~1.5MB of input traffic instead of ~9MB; k-projection
    chains run one tile ahead of v-projection chains so the in-order PE
    never stalls on the later-arriving v-weight chunks.
  - gathers land in TWO half-tiles (j 0-15 / j 16-31) of 4 sub-gathers
    on 4 SWDGE queues, so the qk compute of a tile starts when half the
    bytes have arrived; tiles 0/1 gather in 8 finer subs so the first
    half-tile drains across all 4 rings, shortening the phase-1 ->
    phase-2 bubble.
  - qk d-reduction: full bf16 binary tree (every level runs the DVE
    2x_1p packed mode; tensor_reduce has no fast mode and was 1x).
  - pair weights are host-expanded to (KNN*H) bf16 so logits+weights is
    one contiguous add; logits/logw staged bf16.
  - v-side: (exp x v) product + ONE tree level on DVE, then the
    remaining j-sum runs as identity-lhsT matmuls accumulating into
    PSUM f32 on the mostly-idle PE (identity-matmul = copy-accumulate).
    Offloading MORE than this backfires: heavy concurrent PE activity
    trips the chip activity throttle (HAM k=4/8) and slows every
    engine ~20% (measured on the 32-matmul variant).
  - exp is duplicated x2 as adjacent pairs (ACT) so the v-weighting
    multiply keeps an innermost step-1 pair dim (2x_1p).
  - output staged bf16 and stored as bit-packed f32 words (the axon
    result path mis-decodes bf16 outputs); host unpacks bits.

k/v biases never applied on-device: q.bk cancels in softmax; bv folds
into bo' = bo + bv @ Wo host-side. SCALE folded into Wq/bq. Host sorts
each query's neighbor list (gather locality) and permutes/expands the
pair weights to match.

Known non-wins (measured): SWDGE prepare_only+trigger_dma for the
first tiles produced partial-table gathers (tile-0 corruption) — the
deferred kv_dram dep did not gate the trigger correctly; den via ACT
exp accum_out costs 280ns/accumulator-read (slower than a DVE reduce);
k/v split gathers double desc-gen; PSUM cannot be DMA'd directly.
"""

import sys

import numpy as np

sys.path.insert(0, "/opt/trn_rl_repo")

from contextlib import ExitStack

import ml_dtypes

import concourse.bass as bass
import concourse.tile as tile
from concourse import bacc, masks, mybir
from concourse.bass_utils import run_bass_kernel_spmd

F32 = mybir.dt.float32
BF16 = mybir.dt.bfloat16
I16 = mybir.dt.int16
AF = mybir.ActivationFunctionType
OP = mybir.AluOpType

B, HW, NTGT, C = 2, 4096, 4096, 512
H, KNN = 8, 32
DH = C // H
SCALE = DH ** -0.5
P = 128
QL = HW // 4            # queries per core (1024)
NT = QL // P            # query tiles per core (8)
NTT = NTGT // P         # target tiles (32)
CK = C // P             # contraction chunks (4)
ROW = 2 * C             # kv row elems (1024 bf16 = 2KB)
NSUB = 4                # sub-gathers per query tile
JSUB = KNN // NSUB      # neighbors per sub-gather
JH = KNN // 2           # neighbors per compute half


def build_program():
    nc = bacc.Bacc("TRN2", target_bir_lowering=False, debug=False,
                   num_devices=8, num_swdge_queues=4)

    srcT = nc.dram_tensor("srcT", (C, QL), BF16, kind="ExternalInput").ap()
    tgtT = nc.dram_tensor("tgtT", (C, NTGT), BF16, kind="ExternalInput").ap()
    wq = nc.dram_tensor("wq", (C, C), BF16, kind="ExternalInput").ap()
    wk = nc.dram_tensor("wk", (C, C), BF16, kind="ExternalInput").ap()
    wv = nc.dram_tensor("wv", (C, C), BF16, kind="ExternalInput").ap()
    wo = nc.dram_tensor("wo", (C, C), BF16, kind="ExternalInput").ap()
    bq = nc.dram_tensor("bq", (1, C), BF16, kind="ExternalInput").ap()
    bo = nc.dram_tensor("bo", (1, C), BF16, kind="ExternalInput").ap()
    idxw = nc.dram_tensor("idxw", (NT, P, KNN * P // 16), I16,
                          kind="ExternalInput").ap()
    wts = nc.dram_tensor("wts", (QL, KNN * H), BF16,
                         kind="ExternalInput").ap()
    # output staged bf16 and stored as raw bit-packed f32 words (the
    # axon result path mis-decodes bf16 outputs; bytes are bytes).
    out = nc.dram_tensor("out", (QL, C // 2), F32,
                         kind="ExternalOutput").ap()

    with tile.TileContext(nc) as tc, ExitStack() as ctx:
        tp = lambda name, bufs, **kw: ctx.enter_context(
            tc.tile_pool(name=name, bufs=bufs, **kw))

        cpool = tp("consts", 1)
        dram = tp("dram", 1, space="DRAM")
        kv_dram = dram.tile([NTGT, ROW], BF16)
        small = tp("small", 2)
        gat = tp("gather", 2)

        # ---- early: idx loads for tiles 0/1 (gen inputs), then phase-1
        # chunk loads so the kv projection starts after chunk 0 ----
        st = {}
        for t in range(2):
            idx_sb = small.tile([P, KNN * P // 16], I16, tag="idx")
            nc.sync.dma_start(idx_sb[:], idxw[t, :, :])
            st[t] = {"idx": idx_sb}

        p1_scope = ExitStack()
        p1w = p1_scope.enter_context(tc.tile_pool(name="p1w", bufs=1))
        wkv_c, tgt_c = [], []
        for c in range(CK):
            wkv = p1w.tile([P, ROW], BF16, tag=f"wkv{c}")
            tgc = p1w.tile([P, NTGT], BF16, tag=f"tgt{c}")
            wkv_c.append(wkv)
            tgt_c.append(tgc)
        # load order: everything the FIRST k-half accumulation chains
        # need (k-weight chunks + tgt quarter 0), then the rest — the
        # kv projection starts after ~1.5MB instead of ~9MB.
        for c in range(CK):
            nc.sync.dma_start(wkv_c[c][:, 0:C], wk[c * P:(c + 1) * P, :])
            nc.sync.dma_start(tgt_c[c][:, 0:NTGT // 4],
                              tgtT[c * P:(c + 1) * P, 0:NTGT // 4])
        for c in range(CK):
            nc.sync.dma_start(wkv_c[c][:, C:ROW], wv[c * P:(c + 1) * P, :])
        for j in range(1, 4):
            for c in range(CK):
                nc.sync.dma_start(
                    tgt_c[c][:, j * NTGT // 4:(j + 1) * NTGT // 4],
                    tgtT[c * P:(c + 1) * P,
                         j * NTGT // 4:(j + 1) * NTGT // 4])

        ident = cpool.tile([P, P], BF16, tag="ident")
        masks.make_identity(nc, ident[:])
        ones = cpool.tile([1, P], BF16, tag="ones")
        nc.gpsimd.memset(ones[:], 1.0)

        # remaining resident inputs, queued behind the phase-1 chunks
        wq_sb = cpool.tile([P, CK * C], BF16, tag="wq")
        wo_sb = cpool.tile([P, CK * C], BF16, tag="wo")
        bq_sb = cpool.tile([1, C], BF16, tag="bq")
        bo_sb = cpool.tile([1, C], BF16, tag="bo")
        for c in range(CK):
            nc.sync.dma_start(wq_sb[:, c * C:(c + 1) * C],
                              wq[c * P:(c + 1) * P, :])
            nc.sync.dma_start(wo_sb[:, c * C:(c + 1) * C],
                              wo[c * P:(c + 1) * P, :])
        nc.sync.dma_start(bq_sb[:1, :], bq[:, :])
        nc.sync.dma_start(bo_sb[:1, :], bo[:, :])

        def emit_gathers(t, kvgs, idx_sb, nsub=NSUB):
            """Sub-gathers on separate SWDGE queues into two half-tiles
            (kvgs = [j 0..15, j 16..31]) so stage_bc's first half only
            waits for the first half of the gathered bytes."""
            js = KNN // nsub
            per = JH // js  # subs per half-tile
            for sub in range(nsub):
                kvg = kvgs[sub // per]
                lo = (sub % per) * js * ROW
                nc.gpsimd.dma_gather(
                    kvg[:, lo:lo + js * ROW]
                        .rearrange("p (j d) -> p j d", j=js),
                    kv_dram[:, :],
                    idx_sb[:, sub * js * P // 16:(sub + 1) * js * P // 16],
                    num_idxs=js * P,
                    num_idxs_reg=js * P,
                    elem_size=ROW,
                    single_packet=False,
                    queue_num=sub % NSUB,
                )

        # ---- phase 1: k/v projection of the FULL target table.
        # k-chains run one tile ahead of v-chains: the v-weight chunks
        # load after the k chunks, and the in-order PE would otherwise
        # stall on tile 0's v-chain instead of running tile 1's k-chain.
        with tc.tile_pool(name="p1psum", bufs=2, space="PSUM") as p1ps, \
             tc.tile_pool(name="p1out", bufs=3) as p1out:
            pk = {}

            def k_chain(t):
                pskv = p1ps.tile([P, ROW], F32, tag="pskv")
                for c in range(CK):
                    nc.tensor.matmul(pskv[:, 0:C],
                                     tgt_c[c][:, t * P:(t + 1) * P],
                                     wkv_c[c][:, 0:C],
                                     start=(c == 0), stop=(c == CK - 1))
                pk[t] = pskv

            def v_chain(t):
                pskv = pk.pop(t)
                for c in range(CK):
                    nc.tensor.matmul(pskv[:, C:ROW],
                                     tgt_c[c][:, t * P:(t + 1) * P],
                                     wkv_c[c][:, C:ROW],
                                     start=(c == 0), stop=(c == CK - 1))
                kv_sb = p1out.tile([P, ROW], BF16, tag="kv")
                nc.scalar.copy(kv_sb[:], pskv[:])
                nc.sync.dma_start(kv_dram[t * P:(t + 1) * P, :], kv_sb[:])

            for i in range(NTT + 1):
                if i < NTT:
                    k_chain(i)
                if i >= 1:
                    v_chain(i - 1)
        p1_scope.close()

        # prepare tile 0/1 gather descriptors. Emitted AFTER the kv
        # writes so the preps carry the kv_dram RAW deps — Tile defers
        # those to the trigger, so desc-gen itself still runs on the
        # (otherwise idle) gpsimd engine during phase 1, and the DMA
        # fires right when the kv table lands.
        for t in range(2):
            kvga = gat.tile([P, JH * ROW], BF16, tag="kvga")
            kvgb = gat.tile([P, JH * ROW], BF16, tag="kvgb")
            emit_gathers(t, [kvga, kvgb], st[t]["idx"], nsub=8)
            st[t]["kvg"] = [kvga, kvgb]

        # ---- phase 2 pools ----
        qps = tp("qpsum", 2, space="PSUM")
        tps = tp("tpsum", 2, space="PSUM")
        ops_pool = tp("opsum", 2, space="PSUM")
        vps = tp("vsum", 2, space="PSUM")
        small3 = tp("small3", 3)
        scr = tp("scratch", 1)
        outp = tp("outstage", 2)

        def stage_a(t, emit_gather=True):
            """q-proj + idx/wts loads + gather gen (Pool)."""
            s = st.get(t, {})
            srcq = small.tile([P, CK * P], BF16, tag="srcq")
            for c in range(CK):
                nc.sync.dma_start(srcq[:, c * P:(c + 1) * P],
                                  srcT[c * P:(c + 1) * P,
                                       t * P:(t + 1) * P])
            psq = qps.tile([P, C], F32, tag="psq")
            for c in range(CK):
                nc.tensor.matmul(
                    psq[:], srcq[:, c * P:(c + 1) * P],
                    wq_sb[:, c * C:(c + 1) * C],
                    start=(c == 0), stop=False)
            nc.tensor.matmul(psq[:], ones[:1, :], bq_sb[:1, :],
                             start=False, stop=True)
            q_sb = small.tile([P, C], BF16, tag="q")
            nc.scalar.copy(q_sb[:], psq[:])
            s["q"] = q_sb

            w_sb = small.tile([P, KNN * H], BF16, tag="w")
            nc.sync.dma_start(w_sb[:], wts[t * P:(t + 1) * P, :])
            s["w"] = w_sb

            if emit_gather:
                idx_sb = small.tile([P, KNN * P // 16], I16, tag="idx")
                nc.sync.dma_start(idx_sb[:], idxw[t, :, :])
                kvga = gat.tile([P, JH * ROW], BF16, tag="kvga")
                kvgb = gat.tile([P, JH * ROW], BF16, tag="kvgb")
                emit_gathers(t, [kvga, kvgb], idx_sb)
                s["kvg"] = [kvga, kvgb]
                s["idx"] = idx_sb
            st[t] = s
            return s

        def qk_half(s, jh):
            """qk product + full bf16 d-tree for one j-half -> logits,
            then logits+w add and exp (x2, adjacent pairs)."""
            kvg3 = s["kvg"][jh][:].rearrange("p (j d) -> p j d", j=JH)
            kh = kvg3[:, :, 0:C]
            qb = s["q"][:].unsqueeze(1).broadcast_to([P, JH, C])
            prod = scr.tile([P, JH * C], BF16, tag="prod16")
            nc.vector.tensor_tensor(
                prod[:].rearrange("p (j d) -> p j d", j=JH), kh, qb,
                op=OP.mult)
            vt_in, ew = prod, DH
            for tag in ("t32h", "t16h", "t8h", "t4h", "t2h"):
                ew //= 2
                vt = scr.tile([P, JH * H * ew], BF16, tag=tag)
                a = vt_in[:].rearrange("p (j h e) -> p j h e", j=JH, h=H)
                nc.vector.tensor_tensor(
                    vt[:].rearrange("p (j h e) -> p j h e", j=JH, h=H),
                    a[:, :, :, 0:ew], a[:, :, :, ew:2 * ew], op=OP.add)
                vt_in = vt
            t2v = vt_in[:].rearrange("p (j h e) -> p j h e", j=JH, h=H)
            logits = s["logits"]
            nc.vector.tensor_tensor(
                logits[:, jh * JH * H:(jh + 1) * JH * H]
                    .rearrange("p (j h) -> p j h", j=JH),
                t2v[:, :, :, 0:1], t2v[:, :, :, 1:2], op=OP.add)
            logw = scr.tile([P, JH * H], BF16, tag=f"logw{jh}")
            nc.vector.tensor_tensor(
                logw[:], logits[:, jh * JH * H:(jh + 1) * JH * H],
                s["w"][:, jh * JH * H:(jh + 1) * JH * H], op=OP.add)
            # exp duplicated x2 (adjacent pairs) so the v-weighting's in2
            # has an innermost step-1 pair dim -> DVE 2x_1P packing.
            exd3 = s["exd"][:].rearrange("p (jh t) -> p jh t", t=2)
            hh = jh * JH * H
            nc.scalar.activation(exd3[:, hh:hh + JH * H, 0], logw[:], AF.Exp)
            nc.scalar.activation(exd3[:, hh:hh + JH * H, 1], logw[:], AF.Exp)

        def v_half(s, jh, vsum):
            """(exp x v) products + one DVE tree level; the remaining
            j-sum is 8 identity-lhsT matmuls accumulating into PSUM f32
            (identity-matmul = copy-accumulate on the idle PE)."""
            kvg3 = s["kvg"][jh][:].rearrange("p (j d) -> p j d", j=JH)
            exd4 = s["exd"][:].rearrange("p (j h t) -> p j h t", j=KNN, h=H)
            vh = kvg3[:, :, C:ROW] \
                .rearrange("p j (h e t) -> p j h e t", h=H, t=2)
            exb = (exd4[:, jh * JH:(jh + 1) * JH, :, :]
                   .unsqueeze(3).broadcast_to([P, JH, H, DH // 2, 2]))
            vprod = scr.tile([P, JH * C], BF16, tag="prod16")
            nc.vector.tensor_tensor(
                vprod[:].rearrange("p (j h e t) -> p j h e t",
                                   j=JH, h=H, t=2),
                vh, exb, op=OP.mult)
            jw = JH // 2
            vpart = scr.tile([P, jw * C], BF16, tag=f"vpart{jh}")
            a = vprod[:].rearrange("p (j d) -> p j d", d=C)
            nc.vector.tensor_tensor(
                vpart[:].rearrange("p (j d) -> p j d", d=C),
                a[:, 0:jw, :], a[:, jw:2 * jw, :], op=OP.add)
            vp3 = vpart[:].rearrange("p (j d) -> p j d", d=C)
            for j in range(jw):
                nc.tensor.matmul(vsum[:], ident[:], vp3[:, j, :],
                                 start=(jh == 0 and j == 0),
                                 stop=(jh == 1 and j == jw - 1))

        def stage_bc(s):
            """k-half0 | k-half1 (exp0 on ACT underneath) | v-half0
            (exp1 underneath) | v-half1; den/rec on the side."""
            logits = small.tile([P, KNN * H], BF16, tag="logits")
            exd = small.tile([P, KNN * H * 2], BF16, tag="exd")
            s["logits"], s["exd"] = logits, exd
            qk_half(s, 0)
            qk_half(s, 1)
            vsum = vps.tile([P, C], F32, tag="vsum")
            v_half(s, 0, vsum)
            v_half(s, 1, vsum)
            exd3 = exd[:].rearrange("p (jh t) -> p jh t", t=2)
            den = small3.tile([P, H], F32, tag="den")
            nc.vector.tensor_reduce(
                den[:], exd3[:, :, 0].rearrange("p (j h) -> p h j", j=KNN),
                axis=mybir.AxisListType.X, op=OP.add)
            rec = small3.tile([P, H], F32, tag="rec")
            nc.vector.reciprocal(rec[:], den[:])
            s["vsum"], s["rec"] = vsum, rec

        def stage_d(s, t):
            """per-head 1/den scale + out projection + store."""
            ao = small.tile([P, C], BF16, tag="ao")
            for h in range(H):
                nc.scalar.activation(
                    ao[:, h * DH:(h + 1) * DH],
                    s["vsum"][:, h * DH:(h + 1) * DH],
                    AF.Copy, scale=s["rec"][:, h:h + 1])
            aoT_ps = tps.tile([P, C], BF16, tag="aoT")
            for c in range(CK):
                nc.tensor.transpose(aoT_ps[:, c * P:(c + 1) * P],
                                    ao[:, c * P:(c + 1) * P], ident[:])
            aoT = small.tile([P, C], BF16, tag="aoTsb")
            nc.scalar.copy(aoT[:], aoT_ps[:])
            ops = ops_pool.tile([P, C], F32, tag="ops")
            for c in range(CK):
                nc.tensor.matmul(ops[:], aoT[:, c * P:(c + 1) * P],
                                 wo_sb[:, c * C:(c + 1) * C],
                                 start=(c == 0), stop=False)
            nc.tensor.matmul(ops[:], ones[:1, :], bo_sb[:1, :],
                             start=False, stop=True)
            o_sb = outp.tile([P, C], BF16, tag="osb")
            nc.scalar.copy(o_sb[:], ops[:])
            nc.sync.dma_start(out[t * P:(t + 1) * P, :],
                              o_sb[:].bitcast(F32))

        for i in range(NT + 1):
            if i < NT:
                stage_a(i, emit_gather=(i >= 2))
            if i >= 1:
                t = i - 1
                stage_bc(st[t])
                stage_d(st[t], t)
                del st[t]

    nc.compile()
    return nc


def _wrap_indices(idx_t):
    """(128, KNN) sorted idx -> [128, 256] int16 wrap (j-major flat,
    16-wrapped, replicated across the 8 gpsimd cores)."""
    flat = idx_t.T.reshape(-1)                      # i = j*128 + q
    wr = flat.reshape(-1, 16).T.astype(np.int16)    # [16, KNN*P/16]
    return np.tile(wr, (8, 1))


_NC_CACHE = {}


def _get_program():
    if "nc" not in _NC_CACHE:
        _NC_CACHE["nc"] = build_program()
    return _NC_CACHE["nc"]


def make_in_maps(src, tgt, indices, weights, Wq, bq, Wk, bk, Wv, bv, Wo, bo):
    f32, bf16 = np.float32, ml_dtypes.bfloat16
    src = np.asarray(src, f32)
    tgt = np.asarray(tgt, f32)
    weights = np.asarray(weights, f32)
    wqs = (np.asarray(Wq, f32) * np.float32(SCALE)).astype(bf16)
    bqs = (np.asarray(bq, f32) * np.float32(SCALE)).astype(bf16)
    wk_b = np.asarray(Wk, f32).astype(bf16)
    wv_b = np.asarray(Wv, f32).astype(bf16)
    wo_b = np.asarray(Wo, f32).astype(bf16)
    # v-bias passes through the softmax (weights sum to 1): fold into bo.
    # The k-bias shifts all of a query's logits equally -> cancels in
    # softmax and is dropped entirely.
    bo_b = (np.asarray(bo, f32)
            + np.asarray(bv, f32) @ np.asarray(Wo, f32)).astype(bf16)

    idx_all = np.asarray(indices)
    in_maps = []
    for core in range(8):
        b, s = divmod(core, 4)
        q0 = s * QL
        idx_b = idx_all[b, q0:q0 + QL]              # (QL, KNN)
        w_b = weights[b, q0:q0 + QL]
        order = np.argsort(idx_b, axis=1, kind="stable")
        idx_s = np.take_along_axis(idx_b, order, axis=1)
        w_s = np.take_along_axis(w_b, order, axis=1)
        # expand pair weights over heads (j-major, h fastest) so the
        # logits+weights add is a contiguous tensor_tensor.
        w_exp = np.repeat(w_s, H, axis=1).astype(bf16)
        idxw_c = np.empty((NT, P, KNN * P // 16), np.int16)
        for t in range(NT):
            idxw_c[t] = _wrap_indices(idx_s[t * P:(t + 1) * P])
        m = {
            "srcT": np.ascontiguousarray(src[b, q0:q0 + QL].T).astype(bf16),
            "tgtT": np.ascontiguousarray(tgt[b].T).astype(bf16),
            "wq": wqs, "wk": wk_b, "wv": wv_b, "wo": wo_b,
            "bq": bqs.reshape(1, C), "bo": bo_b.reshape(1, C),
            "idxw": idxw_c,
            "wts": np.ascontiguousarray(w_exp),
        }
        in_maps.append(m)
    return in_maps


def kernel(src, tgt, indices, weights, Wq, bq, Wk, bk, Wv, bv, Wo, bo):
    nc = _get_program()
    in_maps = make_in_maps(src, tgt, indices, weights,
                           Wq, bq, Wk, bk, Wv, bv, Wo, bo)
    res = run_bass_kernel_spmd(nc, in_maps, core_ids=list(range(8)))
    out = np.empty((B, HW, C), np.float32)
    for core in range(8):
        b, s = divmod(core, 4)
        raw = np.ascontiguousarray(res.results[core]["out"])
        u16 = raw.view(np.uint16).reshape(QL, C)
        out[b, s * QL:(s + 1) * QL] = (
            u16.astype(np.uint32) << 16).view(np.float32)
    return out
